# revision 17
# baseline (speedup 1.0000x reference)
"""Trainium2 Bass kernel for nn_AttributedEncoder (GNN attribute message passing).

Strategy (8 NeuronCores, SPMD, no collectives):
  - Host shards EDGES by destination node window (128-node blocks). Global
    windows are distributed to cores snake-wise by edge count so per-core
    totals and per-slot order statistics align; each core's windows are
    processed in sorted (desc count) slot order under a SHARED tile schedule
    T[j] = max_k ceil(count_kj/128). Host un-permutes the output blocks.
  - Math refactor: with s_e = exp(lrelu(ent_s[h] + att_s[att])) the output is
       to_feats[n] = (1/rowsum[n]) * [agg_att | agg_val] @ [W1; W2]
    where agg_* are s-weighted per-node sums of gathered rows, so the
    softmax division becomes a per-NODE scale after aggregation.
  - Per-edge rows come from SWDGE dma_gather (int16 indices) in ONE att call
    and ONE val call per super-block (4 windows) to amortize the ~1us fixed
    SWDGE prep cost. val_feats is REMAPPED per core to its distinct ids
    (~31.5k < 32767) so indices fit int16 with no val-class fragmentation.
  - att table is extended host-side to bf16 rows [attf(256) | 1.0 | hi | lo]
    (768B, 256B-aligned): col 256 is a constant-1 column that makes the att
    matmul also produce the rowsum; cols 257/258 are att_s split hi/lo bf16
    riders (computed on device from f32 attf for fp32-rank precision).
  - Per tile (128 edges): DVE does just TWO fused scalar_tensor_tensor ops:
      xe  = sum_n (iota==hrel)*entb  (accum_out)     [score gather]
      S   = (iota==hrel)*sv          (bf16)          [scored selection]
    with the lrelu+exp applied per-window on the Scalar engine (Prelu+Exp
    share one activation table: no table reloads).
  - PE: per tile 2 matmuls (S stationary): agg_att[n,257] and agg_val[n,256]
    in separate PSUM banks (independent start/stop chains). Finalize per
    window: cast (Scalar), 4 PE transposes, cast, 4 matmuls vs W, then a
    fused scale+add and ELU (min / Exp / fused max-sub).
"""
import os
import sys
import types

import numpy as np

sys.path.insert(0, "/opt/trn_rl_repo")
if "/root/.axon_site" not in sys.path:
    sys.path.insert(0, "/root/.axon_site")


def _install_trace_hook():
    try:
        import antenv
        if "antenv.axon_hooks" in sys.modules:
            return
        from trn_agent_boot.trn_boot import _ntff_profile_via_ctypes

        hook = _ntff_profile_via_ctypes("/opt/axon/libaxon_pjrt.so")
        mod = types.ModuleType("antenv.axon_hooks")
        mod.get_axon_ntff_profile_hook = lambda: hook
        mod.set_axon_ntff_profile_hook = lambda h: None
        sys.modules["antenv.axon_hooks"] = mod
        antenv.axon_hooks = mod
    except Exception:
        pass


_install_trace_hook()

from concourse import bass, mybir, tile  # noqa: E402
from concourse import library_config  # noqa: E402
from concourse import bass_utils as _bu  # noqa: E402
from concourse.library_overlay import lower_extended_insts  # noqa: E402
from concourse.masks import make_identity  # noqa: E402
from concourse.mybir import AxisListType, AluOpType, ActivationFunctionType  # noqa: E402

_bu.upload_artifacts = lambda tmpdir: f"file://{tmpdir}"

P = 128
D = 256
N_ENT = 50000
N_ATT = 2000
N_VAL = 100000
N_CORES = 8
NODES_PER_CORE = 6272          # 49 windows of 128; 8*6272 = 50176 >= 50000
NW = NODES_PER_CORE // P       # 49
GW = N_CORES * NW              # 392 global windows
SBW = 2                        # windows per super-block
NATT_PAD = 2048
AEXT = 384                     # bf16: attf(256) | one | s_hi | s_lo | pad
VAL_ROWS = 32768               # per-core remapped val table rows (int16-safe)
PAD_HREL = 200.0


def legalize_waits(nc, max_engine_waits=1):
    """Hoist excess sync waits onto standalone EventSemaphore instructions on
    the op's own sequencer engine (queue DMAs encode at most one wait)."""
    wid = 0
    for b in nc.m.functions[0].blocks:
        newinsts = []
        for inst in b.instructions:
            si = getattr(inst, "sync_info", None)
            ow = list(si.on_wait) if si and si.on_wait else []
            qname = getattr(inst, "queue", None)
            is_q = bool(qname) or type(inst).__name__ in (
                "InstDMAGatherAnt", "InstDMAScatterAddAnt", "InstDMACopy", "InstNoOp")
            limit = 1 if is_q else max_engine_waits
            if len(ow) > limit:
                while len(ow) > limit:
                    w, ow = ow[0], ow[1:]
                    es = mybir.InstEventSemaphore(
                        name=f"WAITC-{wid}", engine=inst.engine, ins=[], outs=[])
                    wid += 1
                    es.sync_info = mybir.SyncInfo(on_wait=[w], on_update=[])
                    try:
                        nc.register_instruction(es)
                    except Exception:
                        pass
                    newinsts.append(es)
                si.on_wait = ow
            newinsts.append(inst)
        b.instructions = newinsts


def reassign_swdge_queues(nc, n_queues=4):
    """Tile assigns DMASW sem lanes in final instruction order (mod 8); the
    sim locks each lane to one SWDGE queue. Renumber queue_num in the same
    order so lane L always pairs with queue L % n_queues."""
    cnt = 0
    for b in nc.m.functions[0].blocks:
        for inst in b.instructions:
            if type(inst).__name__ in ("InstDMAGatherAnt", "InstDMAScatterAddAnt"):
                inst.queue_num = cnt % n_queues
                cnt += 1
            elif type(inst).__name__ == "InstDMACopy" and \
                    getattr(inst, "queue", "") == "qPoolDynamic":
                cnt += 1


def _pack16(flat):
    """dma_gather index layout: idxs_ap[p, s] = flat[s*16 + p], first-16-row
    block replicated across the 8 Q7 core groups (128 partitions)."""
    n = len(flat)
    assert n % 16 == 0
    blk = np.asarray(flat, dtype=np.int16).reshape(n // 16, 16).T
    return np.tile(blk, (8, 1))


def _route(attribute_triples):
    """Window permutation + per-core edge routing.

    Returns:
      sched: shared schedule dict (T, sbs, tiles->window map, call layout)
      cores: per-core dict with window list (global window ids, slot order)
             and per-slot-tile edge arrays (hrel, vloc via remap, att, vuniq)
    """
    h = np.asarray(attribute_triples[:, 0], dtype=np.int64)
    val = np.asarray(attribute_triples[:, 1], dtype=np.int64)
    att = np.asarray(attribute_triples[:, 2], dtype=np.int64)
    gw = h // P
    cnt = np.bincount(gw, minlength=GW)

    # snake-deal windows (desc by count) to cores
    order = np.argsort(-cnt, kind="stable")
    wins_of_core = [[] for _ in range(N_CORES)]
    for r, w in enumerate(order):
        k = r % N_CORES if (r // N_CORES) % 2 == 0 else N_CORES - 1 - (r % N_CORES)
        wins_of_core[k].append(w)
    # per-core slot order: desc by count
    for k in range(N_CORES):
        wl = wins_of_core[k]
        wl.sort(key=lambda w: -cnt[w])
        assert len(wl) == NW

    # shared tile schedule
    T = np.ones(NW, dtype=np.int64)
    for k in range(N_CORES):
        for j, w in enumerate(wins_of_core[k]):
            T[j] = max(T[j], -(-cnt[w] // P))
    t0_of = np.concatenate([[0], np.cumsum(T)])
    NT = int(t0_of[-1])

    # super-blocks of window-slots
    sbs = [list(range(s, min(s + SBW, NW))) for s in range(0, NW, SBW)]
    sb_t0 = [int(t0_of[sb[0]]) for sb in sbs]
    sb_nt = [int(t0_of[sb[-1] + 1] - t0_of[sb[0]]) for sb in sbs]

    # per-core edge slot arrays
    edge_of_w = {}
    for e, w in enumerate(gw):
        edge_of_w.setdefault(w, []).append(e)
    cores = []
    for k in range(N_CORES):
        hrelf = np.full((NT, P), PAD_HREL, dtype=np.float32)
        vgl = np.zeros((NT, P), dtype=np.int64)   # global val id (remap later)
        ati = np.zeros((NT, P), dtype=np.int64)
        used = np.zeros((NT, P), dtype=bool)
        for j, w in enumerate(wins_of_core[k]):
            es = edge_of_w.get(w, [])
            assert len(es) <= T[j] * P
            t0 = int(t0_of[j])
            for q, e in enumerate(es):
                t, p = t0 + q // P, q % P
                hrelf[t, p] = h[e] % P
                vgl[t, p] = val[e]
                ati[t, p] = att[e]
                used[t, p] = True
        vuniq, vinv = np.unique(vgl[used], return_inverse=True)
        assert len(vuniq) <= VAL_ROWS, len(vuniq)
        vloc = np.zeros((NT, P), dtype=np.int64)
        vloc[used] = vinv
        cores.append(dict(wins=wins_of_core[k], hrelf=hrelf, vloc=vloc,
                          ati=ati, vuniq=vuniq))
    sched = dict(T=T, t0_of=t0_of, NT=NT, sbs=sbs, sb_t0=sb_t0, sb_nt=sb_nt,
                 max_nt=max(sb_nt))
    return sched, cores


def build_program(sched):
    NT = sched["NT"]
    T, t0_of = sched["T"], sched["t0_of"]
    f32 = mybir.dt.float32
    bf16 = mybir.dt.bfloat16
    nc = bass.Bass(num_swdge_queues=4)
    ent_d = nc.declare_dram_parameter("ent", [P, NW * D], f32, isOutput=False)
    attf_d = nc.declare_dram_parameter("attf", [P, 16 * D], f32, isOutput=False)
    attx_d = nc.declare_dram_parameter("attx", [NATT_PAD, AEXT], bf16, isOutput=False)
    valp_d = nc.declare_dram_parameter("valp", [VAL_ROWS, D], bf16, isOutput=False)
    a1r_d = nc.declare_dram_parameter("a1r", [P, D], f32, isOutput=False)
    a2r_d = nc.declare_dram_parameter("a2r", [P, D], f32, isOutput=False)
    br_d = nc.declare_dram_parameter("br", [P, 1], f32, isOutput=False)
    wts_d = nc.declare_dram_parameter("wts", [P, 4 * D], bf16, isOutput=False)
    iota_d = nc.declare_dram_parameter("iota", [P, P], f32, isOutput=False)
    hrel_d = nc.declare_dram_parameter("hrelf", [P, NT], f32, isOutput=False)
    vidx_d = nc.declare_dram_parameter("vidx", [P, 8 * NT], mybir.dt.int16, isOutput=False)
    aidx_d = nc.declare_dram_parameter("aidx", [P, 8 * NT], mybir.dt.int16, isOutput=False)
    out_d = nc.declare_dram_parameter("out", [NODES_PER_CORE, D], f32, isOutput=True)

    MAXNT = sched["max_nt"]
    MAXTW = int(T.max())

    with tile.TileContext(nc) as tc:
        nc.gpsimd.load_library(library_config.mlp)
        with (
            tc.tile_pool(name="const", bufs=1) as cp,
            tc.tile_pool(name="ent", bufs=1) as ep,
            tc.tile_pool(name="ph0", bufs=2) as php,
            tc.tile_pool(name="ga", bufs=8) as gap,
            tc.tile_pool(name="gv", bufs=8) as gvp,
            tc.tile_pool(name="dump", bufs=2) as dpp,
            tc.tile_pool(name="sc", bufs=4) as scp,
            tc.tile_pool(name="ss", bufs=4) as ssp,
            tc.tile_pool(name="fin", bufs=3) as fip,
            tc.tile_pool(name="aat", bufs=2, space="PSUM") as aap,
            tc.tile_pool(name="avl", bufs=2, space="PSUM") as avp,
            tc.tile_pool(name="ebp", bufs=2, space="PSUM") as ebp,
            tc.tile_pool(name="tpp", bufs=1, space="PSUM") as tpp_p,
            tc.tile_pool(name="opp", bufs=1, space="PSUM") as opp,
        ):
            # ---- constants / parameters to SBUF ----
            ident = cp.tile([P, P], f32, tag="ident")
            make_identity(nc, ident[:])
            ident_bf = cp.tile([P, P], bf16, tag="identb")
            nc.scalar.copy(out=ident_bf[:], in_=ident[:])
            iota_s = cp.tile([P, P], f32, tag="iota")
            nc.sync.dma_start(out=iota_s[:], in_=iota_d[:])
            iota3_s = cp.tile([P, 1, P], f32, tag="iota3")
            nc.sync.dma_start(out=iota3_s[:, 0, :], in_=iota_d[:])
            a1r = cp.tile([P, 1, D], f32, tag="a1r")
            nc.sync.dma_start(out=a1r[:, 0, :], in_=a1r_d[:])
            a2r = cp.tile([P, 1, D], f32, tag="a2r")
            nc.sync.dma_start(out=a2r[:, 0, :], in_=a2r_d[:])
            br = cp.tile([P, 1], f32, tag="br")
            nc.sync.dma_start(out=br[:], in_=br_d[:])
            wts = cp.tile([P, 4 * D], bf16, tag="wts")
            nc.sync.dma_start(out=wts[:], in_=wts_d[:])
            hrel_s = cp.tile([P, NT], f32, tag="hrel")
            nc.sync.dma_start(out=hrel_s[:], in_=hrel_d[:])
            vidx_s = cp.tile([P, 8 * NT], mybir.dt.int16, tag="vidx")
            nc.sync.dma_start(out=vidx_s[:], in_=vidx_d[:])
            aidx_s = cp.tile([P, 8 * NT], mybir.dt.int16, tag="aidx")
            nc.sync.dma_start(out=aidx_s[:], in_=aidx_d[:])

            # ---- phase 0a: ent_s[n] = ent . a1 per window; keep bf16 ent ----
            entsc = cp.tile([P, NW], f32, tag="entsc")
            entres = ep.tile([P, NW, D], bf16, tag="entres")
            NG = (NW + 3) // 4
            for g in range(NG):
                w0, w1 = 4 * g, min(4 * g + 4, NW)
                nwg = w1 - w0
                ent4 = php.tile([P, nwg, D], f32, tag="p0e",
                                padded_shape=[P, 4, D])
                nc.sync.dma_start(out=ent4[:], in_=ent_d[:, w0 * D:w1 * D])
                tmp4 = php.tile([P, nwg, D], f32, tag="p0t",
                                padded_shape=[P, 4, D])
                nc.vector.tensor_tensor(
                    out=tmp4[:], in0=ent4[:],
                    in1=a1r[:].to_broadcast([P, nwg, D]),
                    op=AluOpType.mult)
                nc.vector.reduce_sum(out=entsc[:, w0:w1], in_=tmp4[:],
                                     axis=AxisListType.X)
                nc.scalar.copy(out=entres[:, w0:w1, :], in_=ent4[:])

            # ---- phase 0b: att_s riders (hi/lo bf16) into attx cols 257:259 ----
            for g in range(4):
                att4 = php.tile([P, 4, D], f32, tag="p0a")
                nc.sync.dma_start(out=att4[:], in_=attf_d[:, g * 4 * D:(g + 1) * 4 * D])
                tmpa = php.tile([P, 4, D], f32, tag="p0ta")
                nc.vector.tensor_tensor(
                    out=tmpa[:], in0=att4[:],
                    in1=a2r[:].to_broadcast([P, 4, D]),
                    op=AluOpType.mult)
                s4 = php.tile([P, 4], f32, tag="p0s")
                nc.vector.reduce_sum(out=s4[:], in_=tmpa[:], axis=AxisListType.X)
                nc.vector.tensor_scalar(out=s4[:], in0=s4[:], scalar1=br[:],
                                        scalar2=None, op0=AluOpType.add)
                hilo = php.tile([P, 4, 2], bf16, tag="p0h")
                nc.scalar.copy(out=hilo[:, :, 0], in_=s4[:])
                hif = php.tile([P, 4], f32, tag="p0hf")
                nc.scalar.copy(out=hif[:], in_=hilo[:, :, 0])
                lo4 = php.tile([P, 4], f32, tag="p0l")
                nc.vector.tensor_tensor(out=lo4[:], in0=s4[:], in1=hif[:],
                                        op=AluOpType.subtract)
                nc.scalar.copy(out=hilo[:, :, 1], in_=lo4[:])
                for i in range(4):
                    ch = 4 * g + i
                    nc.scalar.dma_start(
                        out=attx_d[ch * P:(ch + 1) * P, 257:259],
                        in_=hilo[:, i, :])

            # ---- main loop over super-blocks ----
            sbs = sched["sbs"]
            slot_att = {}
            slot_val = {}
            nregs = {}

            def issue_gathers(j):
                t0, tw = int(t0_of[j]), int(T[j])
                ht = (tw + 1) // 2
                sa = gap.tile([P, tw, AEXT], bf16, tag="ga",
                              padded_shape=[P, MAXTW, AEXT])
                sv_ = gvp.tile([P, tw, D], bf16, tag="gv",
                               padded_shape=[P, MAXTW, D])
                # two half-calls per stream; order alternates per window so
                # att/val packets average out across the 4 SWDGE queues
                halves = [(0, ht), (ht, tw)] if ht < tw else [(0, ht)]
                calls = []
                for a, b in halves:
                    calls.append(("att", a, b))
                    calls.append(("val", a, b))
                if j % 2:
                    calls = calls[1::2] + calls[0::2]
                for kind, a, b in calls:
                    n = P * (b - a)
                    if n not in nregs:
                        nregs[n] = nc.gpsimd.to_reg(n)
                    if kind == "att":
                        nc.gpsimd.dma_gather(
                            out_ap=sa[:, a:b, :], in_ap=attx_d[:],
                            idxs_ap=aidx_s[:, 8 * (t0 + a):8 * (t0 + b)],
                            num_idxs=n, num_idxs_reg=nregs[n],
                            elem_size=AEXT, single_packet=False, queue_num=0)
                    else:
                        nc.gpsimd.dma_gather(
                            out_ap=sv_[:, a:b, :], in_ap=valp_d[:],
                            idxs_ap=vidx_s[:, 8 * (t0 + a):8 * (t0 + b)],
                            num_idxs=n, num_idxs_reg=nregs[n],
                            elem_size=D, single_packet=False, queue_num=0)
                slot_att[j] = sa
                slot_val[j] = sv_

            # score phase state per window: (sv_tile, g, j)
            pend = None

            def score_phase(j):
                t0, tw = int(t0_of[j]), int(T[j])
                sa = slot_att[j]
                # entb[p, n] = ent_s[node n of window j]  (PSUM, via PE transpose)
                ebt = ebp.tile([P, 1, P], f32, tag="eb", space="PSUM")
                nc.tensor.transpose(out=ebt[:, 0, :],
                                    in_=entsc[:, j:j + 1].to_broadcast([P, P]),
                                    identity=ident[:])
                # S0[p, ti, n] = (hrel[p, t0+ti] == n), all tiles at once
                S0 = ssp.tile([P, MAXTW, P], bf16, tag="S0")
                hr3 = hrel_s[:, t0:t0 + tw]
                nc.vector.tensor_tensor(
                    out=S0[:, 0:tw, :],
                    in0=hr3.to_broadcast([P, tw, P]),
                    in1=iota3_s[:].to_broadcast([P, tw, P]),
                    op=AluOpType.is_equal)
                xm = dpp.tile([P, MAXTW, P], bf16, tag="xm")
                nc.vector.tensor_tensor(
                    out=xm[:, 0:tw, :], in0=S0[:, 0:tw, :],
                    in1=ebt[:].to_broadcast([P, tw, P]),
                    op=AluOpType.mult)
                xe = scp.tile([P, MAXTW], f32, tag="xe")
                nc.vector.reduce_sum(out=xe[:, 0:tw], in_=xm[:, 0:tw, :],
                                     axis=AxisListType.X)
                # xa = hi + lo riders; xs = xe + xa
                xs = scp.tile([P, MAXTW], f32, tag="xs")
                nc.vector.tensor_tensor(
                    out=xs[:, 0:tw],
                    in0=sa[:, 0:tw, 257], in1=sa[:, 0:tw, 258],
                    op=AluOpType.add)
                nc.vector.tensor_add(out=xs[:, 0:tw], in0=xs[:, 0:tw],
                                     in1=xe[:, 0:tw])
                # sv = exp(lrelu(xs)) = max(exp(xs), exp(0.2*xs))
                e1 = scp.tile([P, MAXTW], f32, tag="e1")
                nc.scalar.activation(out=e1[:, 0:tw], in_=xs[:, 0:tw],
                                     func=ActivationFunctionType.Exp)
                e2 = scp.tile([P, MAXTW], f32, tag="e2")
                nc.scalar.activation(out=e2[:, 0:tw], in_=xs[:, 0:tw],
                                     func=ActivationFunctionType.Exp, scale=0.2)
                sv = scp.tile([P, MAXTW], f32, tag="sv")
                nc.vector.tensor_tensor(out=sv[:, 0:tw], in0=e1[:, 0:tw],
                                        in1=e2[:, 0:tw], op=AluOpType.max)
                return sv, S0

            def mm_phase(sv, S0, j):
                t0, tw = int(t0_of[j]), int(T[j])
                sa, svl = slot_att[j], slot_val[j]
                agg_a = aap.tile([P, 257], f32, tag="aat", space="PSUM")
                agg_v = avp.tile([P, D], f32, tag="avl", space="PSUM")
                S_all = ssp.tile([P, MAXTW, P], bf16, tag="S")
                nc.vector.tensor_tensor(
                    out=S_all[:, 0:tw, :], in0=S0[:, 0:tw, :],
                    in1=sv[:, 0:tw].to_broadcast([P, tw, P]),
                    op=AluOpType.mult)
                for ti in range(tw):
                    nc.tensor.matmul(out=agg_a[:], lhsT=S_all[:, ti, :],
                                     rhs=sa[:, ti, 0:257],
                                     start=(ti == 0), stop=(ti == tw - 1))
                    nc.tensor.matmul(out=agg_v[:], lhsT=S_all[:, ti, :],
                                     rhs=svl[:, ti, :],
                                     start=(ti == 0), stop=(ti == tw - 1))
                # ---- finalize window j ----
                rr = fip.tile([P, 1], f32, tag="rr")
                nc.vector.tensor_scalar(out=rr[:], in0=agg_a[:, 256:257],
                                        scalar1=1e-30, scalar2=None,
                                        op0=AluOpType.max)
                nc.vector.reciprocal(out=rr[:], in_=rr[:])
                aggs = fip.tile([P, 513], bf16, tag="aggs")
                nc.scalar.copy(out=aggs[:, 0:257], in_=agg_a[:])
                nc.scalar.copy(out=aggs[:, 257:513], in_=agg_v[:])
                tpp = tpp_p.tile([P, 4 * P], f32, tag="tp", space="PSUM")
                tms = []
                for q in range(4):
                    c0 = q * P if q < 2 else q * P + 1
                    tm = nc.tensor.matmul(out=tpp[:, q * P:(q + 1) * P],
                                          lhsT=aggs[:, c0:c0 + P],
                                          rhs=ident_bf[:],
                                          start=(q == 0), stop=(q == 3))
                    tm.ins.bass_skip_group_check = True
                    tms.append(tm)
                for q in (1, 2, 3):
                    tile.add_dep_helper(tms[q].ins, tms[0].ins, sync=False,
                                        reason="psum bank-clear order")
                tps = fip.tile([P, 4 * P], bf16, tag="tps")
                nc.scalar.copy(out=tps[:], in_=tpp[:])
                opsum = opp.tile([P, D], f32, tag="op", space="PSUM")
                for q in range(4):
                    nc.tensor.matmul(out=opsum[:],
                                     lhsT=tps[:, q * P:(q + 1) * P],
                                     rhs=wts[:, q * D:(q + 1) * D],
                                     start=(q == 0), stop=(q == 3))
                fin = fip.tile([P, D], f32, tag="fin")
                nc.vector.scalar_tensor_tensor(
                    out=fin[:], in0=opsum[:], scalar=rr[:],
                    in1=entres[:, j, :], op0=AluOpType.mult, op1=AluOpType.add)
                mn = fip.tile([P, D], f32, tag="mn")
                nc.vector.tensor_scalar(out=mn[:], in0=fin[:], scalar1=0.0,
                                        scalar2=None, op0=AluOpType.min)
                ex = fip.tile([P, D], f32, tag="ex")
                nc.scalar.activation(out=ex[:], in_=mn[:],
                                     func=ActivationFunctionType.Exp)
                fo = fip.tile([P, D], f32, tag="fo")
                nc.vector.scalar_tensor_tensor(
                    out=fo[:], in0=ex[:], scalar=-1.0, in1=fin[:],
                    op0=AluOpType.add, op1=AluOpType.max)
                nc.sync.dma_start(out=out_d[j * P:(j + 1) * P, :], in_=fo[:])

            PREF = 7
            pq = []
            for j in range(min(PREF, NW)):
                issue_gathers(j)
            for j in range(NW):
                if j + PREF < NW:
                    issue_gathers(j + PREF)
                sv, S0 = score_phase(j)
                pq.append((sv, S0, j))
                if len(pq) > 2:
                    mm_phase(*pq.pop(0))
            while pq:
                mm_phase(*pq.pop(0))

    lower_extended_insts(nc)
    reassign_swdge_queues(nc)
    legalize_waits(nc)
    return nc


def _per_core_inputs(sched, cores, att_feats, val_feats, ent_feats, a_w, a_b, W):
    import ml_dtypes
    NT = sched["NT"]
    bf16 = ml_dtypes.bfloat16
    attf = np.zeros((NATT_PAD, D), dtype=np.float32)
    attf[:N_ATT] = np.asarray(att_feats, dtype=np.float32)
    attx = np.zeros((NATT_PAD, AEXT), dtype=bf16)
    attx[:, :D] = attf.astype(bf16)
    attx[:, D] = 1.0  # rowsum rider column
    attf_r = np.ascontiguousarray(
        attf.reshape(16, P, D).transpose(1, 0, 2).reshape(P, 16 * D))
    entp = np.zeros((N_CORES * NODES_PER_CORE, D), dtype=np.float32)
    entp[:N_ENT] = np.asarray(ent_feats, dtype=np.float32)
    a_w = np.asarray(a_w, dtype=np.float32)
    a1r = np.tile(a_w[0, :D][None, :], (P, 1)).astype(np.float32)
    a2r = np.tile(a_w[0, D:][None, :], (P, 1)).astype(np.float32)
    br = np.full((P, 1), float(np.asarray(a_b).reshape(-1)[0]), dtype=np.float32)
    w1 = np.asarray(W, dtype=np.float32).astype(bf16)  # [512, 256]
    wts = np.concatenate([w1[q * P:(q + 1) * P, :] for q in range(4)],
                         axis=1)  # [128, 1024]
    iota = np.tile(np.arange(P, dtype=np.float32)[None, :], (P, 1))
    valf = np.asarray(val_feats, dtype=np.float32)

    in_maps = []
    for k in range(N_CORES):
        c = cores[k]
        valp = np.zeros((VAL_ROWS, D), dtype=bf16)
        valp[:len(c["vuniq"])] = valf[c["vuniq"]].astype(bf16)
        entk = np.empty((NODES_PER_CORE, D), dtype=np.float32)
        for j, w in enumerate(c["wins"]):
            entk[j * P:(j + 1) * P] = entp[w * P:(w + 1) * P]
        entk = np.ascontiguousarray(
            entk.reshape(NW, P, D).transpose(1, 0, 2).reshape(P, NW * D))
        vidx = np.zeros((P, 8 * NT), dtype=np.int16)
        aidx = np.zeros((P, 8 * NT), dtype=np.int16)
        vidx[:, :] = _pack16(c["vloc"].reshape(-1))
        aidx[:, :] = _pack16(c["ati"].reshape(-1))
        in_maps.append(dict(
            ent=entk, attf=attf_r, attx=attx, valp=valp, a1r=a1r, a2r=a2r,
            br=br, wts=wts, iota=iota,
            hrelf=np.ascontiguousarray(c["hrelf"].T), vidx=vidx, aidx=aidx,
        ))
    return in_maps


def kernel(attribute_triples, att_feats, val_feats, ent_feats, a_w, a_b, W):
    sched, cores = _route(attribute_triples)
    nc = build_program(sched)
    in_maps = _per_core_inputs(sched, cores, att_feats, val_feats, ent_feats,
                               a_w, a_b, W)
    trace = os.environ.get("KERNEL_TRACE", "0") == "1"
    res = _bu.run_bass_kernel_spmd(nc, in_maps, list(range(N_CORES)), trace=trace)
    if trace and res.exec_time_ns:
        print(f"HW exec time: {res.exec_time_ns} ns")
    out = np.empty((N_CORES * NODES_PER_CORE, D), dtype=np.float32)
    for k in range(N_CORES):
        ok = res.results[k]["out"]
        for j, w in enumerate(cores[k]["wins"]):
            out[w * P:(w + 1) * P] = ok[j * P:(j + 1) * P]
    return np.ascontiguousarray(out[:N_ENT]).astype(np.float32)


# revision 20
# speedup vs baseline: 1.0525x; 1.0525x over previous
"""Trainium2 Bass kernel for nn_AttributedEncoder (GNN attribute message passing).

Strategy (8 NeuronCores, SPMD, no collectives):
  - Host shards EDGES by destination node window (128-node blocks). Global
    windows are distributed to cores snake-wise by edge count so per-core
    totals and per-slot order statistics align; each core's windows are
    processed in sorted (desc count) slot order under a SHARED tile schedule
    T[j] = max_k ceil(count_kj/128). Host un-permutes the output blocks.
  - Math refactor: with s_e = exp(lrelu(ent_s[h] + att_s[att])) the output is
       to_feats[n] = (1/rowsum[n]) * [agg_att | agg_val] @ [W1; W2]
    where agg_* are s-weighted per-node sums of gathered rows, so the
    softmax division becomes a per-NODE scale after aggregation.
  - Per-edge rows come from SWDGE dma_gather (int16 indices) in ONE att call
    and ONE val call per super-block (4 windows) to amortize the ~1us fixed
    SWDGE prep cost. val_feats is REMAPPED per core to its distinct ids
    (~31.5k < 32767) so indices fit int16 with no val-class fragmentation.
  - att table is extended host-side to bf16 rows [attf(256) | 1.0 | hi | lo]
    (768B, 256B-aligned): col 256 is a constant-1 column that makes the att
    matmul also produce the rowsum; cols 257/258 are att_s split hi/lo bf16
    riders (computed on device from f32 attf for fp32-rank precision).
  - Per tile (128 edges): DVE does just TWO fused scalar_tensor_tensor ops:
      xe  = sum_n (iota==hrel)*entb  (accum_out)     [score gather]
      S   = (iota==hrel)*sv          (bf16)          [scored selection]
    with the lrelu+exp applied per-window on the Scalar engine (Prelu+Exp
    share one activation table: no table reloads).
  - PE: per tile 2 matmuls (S stationary): agg_att[n,257] and agg_val[n,256]
    in separate PSUM banks (independent start/stop chains). Finalize per
    window: cast (Scalar), 4 PE transposes, cast, 4 matmuls vs W, then a
    fused scale+add and ELU (min / Exp / fused max-sub).
"""
import os
import sys
import types

import numpy as np

sys.path.insert(0, "/opt/trn_rl_repo")
if "/root/.axon_site" not in sys.path:
    sys.path.insert(0, "/root/.axon_site")


def _install_trace_hook():
    try:
        import antenv
        if "antenv.axon_hooks" in sys.modules:
            return
        from trn_agent_boot.trn_boot import _ntff_profile_via_ctypes

        hook = _ntff_profile_via_ctypes("/opt/axon/libaxon_pjrt.so")
        mod = types.ModuleType("antenv.axon_hooks")
        mod.get_axon_ntff_profile_hook = lambda: hook
        mod.set_axon_ntff_profile_hook = lambda h: None
        sys.modules["antenv.axon_hooks"] = mod
        antenv.axon_hooks = mod
    except Exception:
        pass


_install_trace_hook()

from concourse import bass, mybir, tile  # noqa: E402
from concourse import library_config  # noqa: E402
from concourse import bass_utils as _bu  # noqa: E402
from concourse.library_overlay import lower_extended_insts  # noqa: E402
from concourse.masks import make_identity  # noqa: E402
from concourse.mybir import AxisListType, AluOpType, ActivationFunctionType  # noqa: E402

_bu.upload_artifacts = lambda tmpdir: f"file://{tmpdir}"

P = 128
D = 256
N_ENT = 50000
N_ATT = 2000
N_VAL = 100000
N_CORES = 8
NODES_PER_CORE = 6272          # 49 windows of 128; 8*6272 = 50176 >= 50000
NW = NODES_PER_CORE // P       # 49
GW = N_CORES * NW              # 392 global windows
SBW = 2                        # windows per super-block
NATT_PAD = 2048
AEXT = 384                     # bf16: attf(256) | one | s_hi | s_lo | pad
VAL_ROWS = 32768               # per-core remapped val table rows (int16-safe)
PAD_HREL = 200.0


def legalize_waits(nc, max_engine_waits=1):
    """Hoist excess sync waits onto standalone EventSemaphore instructions on
    the op's own sequencer engine (queue DMAs encode at most one wait)."""
    wid = 0
    for b in nc.m.functions[0].blocks:
        newinsts = []
        for inst in b.instructions:
            si = getattr(inst, "sync_info", None)
            ow = list(si.on_wait) if si and si.on_wait else []
            qname = getattr(inst, "queue", None)
            is_q = bool(qname) or type(inst).__name__ in (
                "InstDMAGatherAnt", "InstDMAScatterAddAnt", "InstDMACopy", "InstNoOp")
            limit = 1 if is_q else max_engine_waits
            if len(ow) > limit:
                while len(ow) > limit:
                    w, ow = ow[0], ow[1:]
                    es = mybir.InstEventSemaphore(
                        name=f"WAITC-{wid}", engine=inst.engine, ins=[], outs=[])
                    wid += 1
                    es.sync_info = mybir.SyncInfo(on_wait=[w], on_update=[])
                    try:
                        nc.register_instruction(es)
                    except Exception:
                        pass
                    newinsts.append(es)
                si.on_wait = ow
            newinsts.append(inst)
        b.instructions = newinsts


def reassign_swdge_queues(nc, n_queues=4):
    """Tile assigns DMASW sem lanes in final instruction order (mod 8); the
    sim locks each lane to one SWDGE queue. Renumber queue_num in the same
    order so lane L always pairs with queue L % n_queues."""
    cnt = 0
    for b in nc.m.functions[0].blocks:
        for inst in b.instructions:
            if type(inst).__name__ in ("InstDMAGatherAnt", "InstDMAScatterAddAnt"):
                inst.queue_num = cnt % n_queues
                cnt += 1
            elif type(inst).__name__ == "InstDMACopy" and \
                    getattr(inst, "queue", "") == "qPoolDynamic":
                cnt += 1


def _pack16(flat):
    """dma_gather index layout: idxs_ap[p, s] = flat[s*16 + p], first-16-row
    block replicated across the 8 Q7 core groups (128 partitions)."""
    n = len(flat)
    assert n % 16 == 0
    blk = np.asarray(flat, dtype=np.int16).reshape(n // 16, 16).T
    return np.tile(blk, (8, 1))


def _route(attribute_triples):
    """Window permutation + per-core edge routing.

    Returns:
      sched: shared schedule dict (T, sbs, tiles->window map, call layout)
      cores: per-core dict with window list (global window ids, slot order)
             and per-slot-tile edge arrays (hrel, vloc via remap, att, vuniq)
    """
    h = np.asarray(attribute_triples[:, 0], dtype=np.int64)
    val = np.asarray(attribute_triples[:, 1], dtype=np.int64)
    att = np.asarray(attribute_triples[:, 2], dtype=np.int64)
    gw = h // P
    cnt = np.bincount(gw, minlength=GW)

    # snake-deal windows (desc by count) to cores
    order = np.argsort(-cnt, kind="stable")
    wins_of_core = [[] for _ in range(N_CORES)]
    for r, w in enumerate(order):
        k = r % N_CORES if (r // N_CORES) % 2 == 0 else N_CORES - 1 - (r % N_CORES)
        wins_of_core[k].append(w)
    # per-core slot order: desc by count
    for k in range(N_CORES):
        wl = wins_of_core[k]
        wl.sort(key=lambda w: -cnt[w])
        assert len(wl) == NW

    # shared tile schedule
    T = np.ones(NW, dtype=np.int64)
    for k in range(N_CORES):
        for j, w in enumerate(wins_of_core[k]):
            T[j] = max(T[j], -(-cnt[w] // P))
    t0_of = np.concatenate([[0], np.cumsum(T)])
    NT = int(t0_of[-1])

    # super-blocks of window-slots
    sbs = [list(range(s, min(s + SBW, NW))) for s in range(0, NW, SBW)]
    sb_t0 = [int(t0_of[sb[0]]) for sb in sbs]
    sb_nt = [int(t0_of[sb[-1] + 1] - t0_of[sb[0]]) for sb in sbs]

    # per-core edge slot arrays
    edge_of_w = {}
    for e, w in enumerate(gw):
        edge_of_w.setdefault(w, []).append(e)
    cores = []
    for k in range(N_CORES):
        hrelf = np.full((NT, P), PAD_HREL, dtype=np.float32)
        vgl = np.zeros((NT, P), dtype=np.int64)   # global val id (remap later)
        ati = np.zeros((NT, P), dtype=np.int64)
        used = np.zeros((NT, P), dtype=bool)
        for j, w in enumerate(wins_of_core[k]):
            es = edge_of_w.get(w, [])
            assert len(es) <= T[j] * P
            t0 = int(t0_of[j])
            for q, e in enumerate(es):
                t, p = t0 + q // P, q % P
                hrelf[t, p] = h[e] % P
                vgl[t, p] = val[e]
                ati[t, p] = att[e]
                used[t, p] = True
        vuniq, vinv = np.unique(vgl[used], return_inverse=True)
        assert len(vuniq) <= VAL_ROWS, len(vuniq)
        vloc = np.zeros((NT, P), dtype=np.int64)
        vloc[used] = vinv
        cores.append(dict(wins=wins_of_core[k], hrelf=hrelf, vloc=vloc,
                          ati=ati, vuniq=vuniq))
    sched = dict(T=T, t0_of=t0_of, NT=NT, sbs=sbs, sb_t0=sb_t0, sb_nt=sb_nt,
                 max_nt=max(sb_nt))
    return sched, cores


def build_program(sched):
    NT = sched["NT"]
    T, t0_of = sched["T"], sched["t0_of"]
    f32 = mybir.dt.float32
    bf16 = mybir.dt.bfloat16
    nc = bass.Bass(num_swdge_queues=4)
    ent_d = nc.declare_dram_parameter("ent", [P, NW * D], f32, isOutput=False)
    attf_d = nc.declare_dram_parameter("attf", [P, 16 * D], f32, isOutput=False)
    attx_d = nc.declare_dram_parameter("attx", [NATT_PAD, AEXT], bf16, isOutput=False)
    valp_d = nc.declare_dram_parameter("valp", [VAL_ROWS, D], bf16, isOutput=False)
    a1r_d = nc.declare_dram_parameter("a1r", [P, D], f32, isOutput=False)
    a2r_d = nc.declare_dram_parameter("a2r", [P, D], f32, isOutput=False)
    br_d = nc.declare_dram_parameter("br", [P, 1], f32, isOutput=False)
    wts_d = nc.declare_dram_parameter("wts", [P, 4 * D], bf16, isOutput=False)
    iota_d = nc.declare_dram_parameter("iota", [P, P], f32, isOutput=False)
    hrel_d = nc.declare_dram_parameter("hrelf", [P, NT], f32, isOutput=False)
    vidx_d = nc.declare_dram_parameter("vidx", [P, 8 * NT], mybir.dt.int16, isOutput=False)
    aidx_d = nc.declare_dram_parameter("aidx", [P, 8 * NT], mybir.dt.int16, isOutput=False)
    out_d = nc.declare_dram_parameter("out", [NODES_PER_CORE, D], f32, isOutput=True)

    MAXNT = sched["max_nt"]
    MAXTW = int(T.max())

    with tile.TileContext(nc) as tc:
        nc.gpsimd.load_library(library_config.mlp)
        with (
            tc.tile_pool(name="const", bufs=1) as cp,
            tc.tile_pool(name="ent", bufs=1) as ep,
            tc.tile_pool(name="ph0", bufs=2) as php,
            tc.tile_pool(name="ga", bufs=6) as gap,
            tc.tile_pool(name="gv", bufs=6) as gvp,
            tc.tile_pool(name="dump", bufs=2) as dpp,
            tc.tile_pool(name="sc", bufs=4) as scp,
            tc.tile_pool(name="ss", bufs=4) as ssp,
            tc.tile_pool(name="fin", bufs=3) as fip,
            tc.tile_pool(name="aat", bufs=2, space="PSUM") as aap,
            tc.tile_pool(name="avl", bufs=2, space="PSUM") as avp,
            tc.tile_pool(name="ebp", bufs=2, space="PSUM") as ebp,
            tc.tile_pool(name="tpp", bufs=1, space="PSUM") as tpp_p,
            tc.tile_pool(name="opp", bufs=1, space="PSUM") as opp,
        ):
            # ---- constants / parameters to SBUF ----
            ident = cp.tile([P, P], f32, tag="ident")
            make_identity(nc, ident[:])
            ident_bf = cp.tile([P, P], bf16, tag="identb")
            nc.scalar.copy(out=ident_bf[:], in_=ident[:])
            iota_s = cp.tile([P, P], f32, tag="iota")
            nc.sync.dma_start(out=iota_s[:], in_=iota_d[:])
            iota3_s = cp.tile([P, 1, P], f32, tag="iota3")
            nc.sync.dma_start(out=iota3_s[:, 0, :], in_=iota_d[:])
            a1r = cp.tile([P, 1, D], f32, tag="a1r")
            nc.sync.dma_start(out=a1r[:, 0, :], in_=a1r_d[:])
            a2r = cp.tile([P, 1, D], f32, tag="a2r")
            nc.sync.dma_start(out=a2r[:, 0, :], in_=a2r_d[:])
            br = cp.tile([P, 1], f32, tag="br")
            nc.sync.dma_start(out=br[:], in_=br_d[:])
            wts = cp.tile([P, 4 * D], bf16, tag="wts")
            nc.sync.dma_start(out=wts[:], in_=wts_d[:])
            hrel_s = cp.tile([P, NT], f32, tag="hrel")
            nc.sync.dma_start(out=hrel_s[:], in_=hrel_d[:])
            vidx_s = cp.tile([P, 8 * NT], mybir.dt.int16, tag="vidx")
            nc.sync.dma_start(out=vidx_s[:], in_=vidx_d[:])
            aidx_s = cp.tile([P, 8 * NT], mybir.dt.int16, tag="aidx")
            nc.sync.dma_start(out=aidx_s[:], in_=aidx_d[:])

            # ---- phase 0a: ent_s[n] = ent . a1 per window; keep bf16 ent ----
            entsc = cp.tile([P, NW], f32, tag="entsc")
            entres = ep.tile([P, NW, D], bf16, tag="entres")
            NG = (NW + 3) // 4
            for g in range(NG):
                w0, w1 = 4 * g, min(4 * g + 4, NW)
                nwg = w1 - w0
                ent4 = php.tile([P, nwg, D], f32, tag="p0e",
                                padded_shape=[P, 4, D])
                nc.sync.dma_start(out=ent4[:], in_=ent_d[:, w0 * D:w1 * D])
                tmp4 = php.tile([P, nwg, D], f32, tag="p0t",
                                padded_shape=[P, 4, D])
                nc.vector.tensor_tensor(
                    out=tmp4[:], in0=ent4[:],
                    in1=a1r[:].to_broadcast([P, nwg, D]),
                    op=AluOpType.mult)
                nc.vector.reduce_sum(out=entsc[:, w0:w1], in_=tmp4[:],
                                     axis=AxisListType.X)
                nc.scalar.copy(out=entres[:, w0:w1, :], in_=ent4[:])

            # ---- phase 0b: att_s riders (hi/lo bf16) into attx cols 257:259 ----
            for g in range(4):
                att4 = php.tile([P, 4, D], f32, tag="p0a")
                nc.sync.dma_start(out=att4[:], in_=attf_d[:, g * 4 * D:(g + 1) * 4 * D])
                tmpa = php.tile([P, 4, D], f32, tag="p0ta")
                nc.vector.tensor_tensor(
                    out=tmpa[:], in0=att4[:],
                    in1=a2r[:].to_broadcast([P, 4, D]),
                    op=AluOpType.mult)
                s4 = php.tile([P, 4], f32, tag="p0s")
                nc.vector.reduce_sum(out=s4[:], in_=tmpa[:], axis=AxisListType.X)
                nc.vector.tensor_scalar(out=s4[:], in0=s4[:], scalar1=br[:],
                                        scalar2=None, op0=AluOpType.add)
                hilo = php.tile([P, 4, 2], bf16, tag="p0h")
                nc.scalar.copy(out=hilo[:, :, 0], in_=s4[:])
                hif = php.tile([P, 4], f32, tag="p0hf")
                nc.scalar.copy(out=hif[:], in_=hilo[:, :, 0])
                lo4 = php.tile([P, 4], f32, tag="p0l")
                nc.vector.tensor_tensor(out=lo4[:], in0=s4[:], in1=hif[:],
                                        op=AluOpType.subtract)
                nc.scalar.copy(out=hilo[:, :, 1], in_=lo4[:])
                for i in range(4):
                    ch = 4 * g + i
                    nc.scalar.dma_start(
                        out=attx_d[ch * P:(ch + 1) * P, 257:259],
                        in_=hilo[:, i, :])

            # ---- main loop over super-blocks ----
            sbs = sched["sbs"]
            slot_att = {}
            slot_val = {}
            nregs = {}

            def issue_gathers(j):
                t0, tw = int(t0_of[j]), int(T[j])
                ht = (tw + 1) // 2
                sa = gap.tile([P, tw, AEXT], bf16, tag="ga",
                              padded_shape=[P, MAXTW, AEXT])
                sv_ = gvp.tile([P, tw, D], bf16, tag="gv",
                               padded_shape=[P, MAXTW, D])
                # two half-calls per stream; order alternates per window so
                # att/val packets average out across the 4 SWDGE queues
                halves = [(0, ht), (ht, tw)] if ht < tw else [(0, ht)]
                calls = []
                for a, b in halves:
                    calls.append(("att", a, b))
                    calls.append(("val", a, b))
                if j % 2:
                    calls = calls[1::2] + calls[0::2]
                for kind, a, b in calls:
                    n = P * (b - a)
                    if n not in nregs:
                        nregs[n] = nc.gpsimd.to_reg(n)
                    if kind == "att":
                        nc.gpsimd.dma_gather(
                            out_ap=sa[:, a:b, :], in_ap=attx_d[:],
                            idxs_ap=aidx_s[:, 8 * (t0 + a):8 * (t0 + b)],
                            num_idxs=n, num_idxs_reg=nregs[n],
                            elem_size=AEXT, single_packet=False, queue_num=0)
                    else:
                        nc.gpsimd.dma_gather(
                            out_ap=sv_[:, a:b, :], in_ap=valp_d[:],
                            idxs_ap=vidx_s[:, 8 * (t0 + a):8 * (t0 + b)],
                            num_idxs=n, num_idxs_reg=nregs[n],
                            elem_size=D, single_packet=False, queue_num=0)
                slot_att[j] = sa
                slot_val[j] = sv_

            # score phase state per window: (sv_tile, g, j)
            pend = None

            def score_phase(j):
                t0, tw = int(t0_of[j]), int(T[j])
                sa = slot_att[j]
                # entb[p, n] = ent_s[node n of window j]  (PSUM, via PE transpose)
                ebt = ebp.tile([P, 1, P], f32, tag="eb", space="PSUM")
                nc.tensor.transpose(out=ebt[:, 0, :],
                                    in_=entsc[:, j:j + 1].to_broadcast([P, P]),
                                    identity=ident[:])
                # S0[p, ti, n] = (hrel[p, t0+ti] == n), all tiles at once
                S0 = ssp.tile([P, MAXTW, P], bf16, tag="S0")
                hr3 = hrel_s[:, t0:t0 + tw]
                nc.vector.tensor_tensor(
                    out=S0[:, 0:tw, :],
                    in0=hr3.to_broadcast([P, tw, P]),
                    in1=iota3_s[:].to_broadcast([P, tw, P]),
                    op=AluOpType.is_equal)
                xm = dpp.tile([P, MAXTW, P], bf16, tag="xm")
                nc.vector.tensor_tensor(
                    out=xm[:, 0:tw, :], in0=S0[:, 0:tw, :],
                    in1=ebt[:].to_broadcast([P, tw, P]),
                    op=AluOpType.mult)
                xe = scp.tile([P, MAXTW], f32, tag="xe")
                nc.vector.reduce_sum(out=xe[:, 0:tw], in_=xm[:, 0:tw, :],
                                     axis=AxisListType.X)
                # xa = hi + lo riders; xs = xe + xa
                xs = scp.tile([P, MAXTW], f32, tag="xs")
                nc.vector.tensor_tensor(
                    out=xs[:, 0:tw],
                    in0=sa[:, 0:tw, 257], in1=sa[:, 0:tw, 258],
                    op=AluOpType.add)
                nc.vector.tensor_add(out=xs[:, 0:tw], in0=xs[:, 0:tw],
                                     in1=xe[:, 0:tw])
                # sv = exp(lrelu(xs)) = max(exp(xs), exp(0.2*xs))
                e1 = scp.tile([P, MAXTW], f32, tag="e1")
                nc.scalar.activation(out=e1[:, 0:tw], in_=xs[:, 0:tw],
                                     func=ActivationFunctionType.Exp)
                e2 = scp.tile([P, MAXTW], f32, tag="e2")
                nc.scalar.activation(out=e2[:, 0:tw], in_=xs[:, 0:tw],
                                     func=ActivationFunctionType.Exp, scale=0.2)
                sv = scp.tile([P, MAXTW], f32, tag="sv")
                nc.vector.tensor_tensor(out=sv[:, 0:tw], in0=e1[:, 0:tw],
                                        in1=e2[:, 0:tw], op=AluOpType.max)
                return sv, S0

            def mm_phase(sv, S0, j):
                t0, tw = int(t0_of[j]), int(T[j])
                sa, svl = slot_att[j], slot_val[j]
                agg_a = aap.tile([P, 257], f32, tag="aat", space="PSUM")
                agg_v = avp.tile([P, D], f32, tag="avl", space="PSUM")
                S_all = ssp.tile([P, MAXTW, P], bf16, tag="S")
                nc.vector.tensor_tensor(
                    out=S_all[:, 0:tw, :], in0=S0[:, 0:tw, :],
                    in1=sv[:, 0:tw].to_broadcast([P, tw, P]),
                    op=AluOpType.mult)
                for ti in range(tw):
                    nc.tensor.matmul(out=agg_a[:], lhsT=S_all[:, ti, :],
                                     rhs=sa[:, ti, 0:257],
                                     start=(ti == 0), stop=(ti == tw - 1))
                    nc.tensor.matmul(out=agg_v[:], lhsT=S_all[:, ti, :],
                                     rhs=svl[:, ti, :],
                                     start=(ti == 0), stop=(ti == tw - 1))
                # ---- finalize window j ----
                rr = fip.tile([P, 1], f32, tag="rr")
                nc.vector.tensor_scalar(out=rr[:], in0=agg_a[:, 256:257],
                                        scalar1=1e-30, scalar2=None,
                                        op0=AluOpType.max)
                nc.vector.reciprocal(out=rr[:], in_=rr[:])
                aggs = fip.tile([P, 513], bf16, tag="aggs")
                nc.scalar.copy(out=aggs[:, 0:257], in_=agg_a[:])
                nc.scalar.copy(out=aggs[:, 257:513], in_=agg_v[:])
                tpp = tpp_p.tile([P, 4 * P], f32, tag="tp", space="PSUM")
                tms = []
                for q in range(4):
                    c0 = q * P if q < 2 else q * P + 1
                    tm = nc.tensor.matmul(out=tpp[:, q * P:(q + 1) * P],
                                          lhsT=aggs[:, c0:c0 + P],
                                          rhs=ident_bf[:],
                                          start=(q == 0), stop=(q == 3))
                    tm.ins.bass_skip_group_check = True
                    tms.append(tm)
                for q in (1, 2, 3):
                    tile.add_dep_helper(tms[q].ins, tms[0].ins, sync=False,
                                        reason="psum bank-clear order")
                tps = fip.tile([P, 4 * P], bf16, tag="tps")
                nc.scalar.copy(out=tps[:], in_=tpp[:])
                opsum = opp.tile([P, D], f32, tag="op", space="PSUM")
                for q in range(4):
                    nc.tensor.matmul(out=opsum[:],
                                     lhsT=tps[:, q * P:(q + 1) * P],
                                     rhs=wts[:, q * D:(q + 1) * D],
                                     start=(q == 0), stop=(q == 3))
                fin = fip.tile([P, D], f32, tag="fin")
                nc.vector.scalar_tensor_tensor(
                    out=fin[:], in0=opsum[:], scalar=rr[:],
                    in1=entres[:, j, :], op0=AluOpType.mult, op1=AluOpType.add)
                mn = fip.tile([P, D], f32, tag="mn")
                nc.scalar.activation(out=mn[:], in_=fin[:],
                                     func=ActivationFunctionType.Relu,
                                     scale=-1.0)
                ex = fip.tile([P, D], f32, tag="ex")
                nc.scalar.activation(out=ex[:], in_=mn[:],
                                     func=ActivationFunctionType.Exp,
                                     scale=-1.0)
                fo = fip.tile([P, D], f32, tag="fo")
                nc.vector.scalar_tensor_tensor(
                    out=fo[:], in0=ex[:], scalar=-1.0, in1=fin[:],
                    op0=AluOpType.add, op1=AluOpType.max)
                nc.sync.dma_start(out=out_d[j * P:(j + 1) * P, :], in_=fo[:])

            PREF = 5
            pq = []
            for j in range(min(PREF, NW)):
                issue_gathers(j)
            for j in range(NW):
                if j + PREF < NW:
                    issue_gathers(j + PREF)
                sv, S0 = score_phase(j)
                pq.append((sv, S0, j))
                if len(pq) > 2:
                    mm_phase(*pq.pop(0))
            while pq:
                mm_phase(*pq.pop(0))

    lower_extended_insts(nc)
    reassign_swdge_queues(nc)
    legalize_waits(nc)
    return nc


def _per_core_inputs(sched, cores, att_feats, val_feats, ent_feats, a_w, a_b, W):
    import ml_dtypes
    NT = sched["NT"]
    bf16 = ml_dtypes.bfloat16
    attf = np.zeros((NATT_PAD, D), dtype=np.float32)
    attf[:N_ATT] = np.asarray(att_feats, dtype=np.float32)
    attx = np.zeros((NATT_PAD, AEXT), dtype=bf16)
    attx[:, :D] = attf.astype(bf16)
    attx[:, D] = 1.0  # rowsum rider column
    attf_r = np.ascontiguousarray(
        attf.reshape(16, P, D).transpose(1, 0, 2).reshape(P, 16 * D))
    entp = np.zeros((N_CORES * NODES_PER_CORE, D), dtype=np.float32)
    entp[:N_ENT] = np.asarray(ent_feats, dtype=np.float32)
    a_w = np.asarray(a_w, dtype=np.float32)
    a1r = np.tile(a_w[0, :D][None, :], (P, 1)).astype(np.float32)
    a2r = np.tile(a_w[0, D:][None, :], (P, 1)).astype(np.float32)
    br = np.full((P, 1), float(np.asarray(a_b).reshape(-1)[0]), dtype=np.float32)
    w1 = np.asarray(W, dtype=np.float32).astype(bf16)  # [512, 256]
    wts = np.concatenate([w1[q * P:(q + 1) * P, :] for q in range(4)],
                         axis=1)  # [128, 1024]
    iota = np.tile(np.arange(P, dtype=np.float32)[None, :], (P, 1))
    valf = np.asarray(val_feats, dtype=np.float32)

    in_maps = []
    for k in range(N_CORES):
        c = cores[k]
        valp = np.zeros((VAL_ROWS, D), dtype=bf16)
        valp[:len(c["vuniq"])] = valf[c["vuniq"]].astype(bf16)
        entk = np.empty((NODES_PER_CORE, D), dtype=np.float32)
        for j, w in enumerate(c["wins"]):
            entk[j * P:(j + 1) * P] = entp[w * P:(w + 1) * P]
        entk = np.ascontiguousarray(
            entk.reshape(NW, P, D).transpose(1, 0, 2).reshape(P, NW * D))
        vidx = np.zeros((P, 8 * NT), dtype=np.int16)
        aidx = np.zeros((P, 8 * NT), dtype=np.int16)
        vidx[:, :] = _pack16(c["vloc"].reshape(-1))
        aidx[:, :] = _pack16(c["ati"].reshape(-1))
        in_maps.append(dict(
            ent=entk, attf=attf_r, attx=attx, valp=valp, a1r=a1r, a2r=a2r,
            br=br, wts=wts, iota=iota,
            hrelf=np.ascontiguousarray(c["hrelf"].T), vidx=vidx, aidx=aidx,
        ))
    return in_maps


def kernel(attribute_triples, att_feats, val_feats, ent_feats, a_w, a_b, W):
    sched, cores = _route(attribute_triples)
    nc = build_program(sched)
    in_maps = _per_core_inputs(sched, cores, att_feats, val_feats, ent_feats,
                               a_w, a_b, W)
    trace = os.environ.get("KERNEL_TRACE", "0") == "1"
    res = _bu.run_bass_kernel_spmd(nc, in_maps, list(range(N_CORES)), trace=trace)
    if trace and res.exec_time_ns:
        print(f"HW exec time: {res.exec_time_ns} ns")
    out = np.empty((N_CORES * NODES_PER_CORE, D), dtype=np.float32)
    for k in range(N_CORES):
        ok = res.results[k]["out"]
        for j, w in enumerate(cores[k]["wins"]):
            out[w * P:(w + 1) * P] = ok[j * P:(j + 1) * P]
    return np.ascontiguousarray(out[:N_ENT]).astype(np.float32)
